# revision 14
# baseline (speedup 1.0000x reference)
"""Trainium2 Bass kernel for a 2-layer GCN (gnn_message_passing).

Reference computation (all f32 inputs):
    h      = relu(adj @ (x @ W1) + b1)        adj: [N, N], x: [N, F]
    logits = adj @ (h @ W2) + b2
    out    = log_softmax(logits, axis=1)       out: [N, C]

Distribution: 1-D row partition over 8 NeuronCores. Core t owns rows
R_t = [t*M, (t+1)*M). adj is symmetric, so adj[:, R_t] (shipped in
natural column-slice layout) doubles as the transposed moving operand
for both layers.

v3 design (vs the f32/bf16 + double-AllGather baseline):
  - adj ships from the HOST as fp8e4m3 in a paired-chunk layout
    [128, K2, 2, M] (8 MB/core instead of 32): DMA lands directly in
    the resident SBUF tile - no staging pools, no on-chip casts. The
    fp8 pairing feeds layer-2's DoubleRow matmuls (0.5 cyc/row) and
    layer-1 streams it as a plain fp8 moving operand (1 cyc/row).
  - the S AllGather is gone: every core computes the FULL S^T = W1^T x
    locally from a replicated bf16 x (fat 512-wide matmuls, 4 ldw per
    512 columns), then converts S^T -> S chunks with DMA-xbar
    transposes (off the PE). S-in-fp8 fails the accuracy budget
    (2.5e-2 > 2e-2 measured), so S stays bf16 and layer-1 runs mixed
    fp8 x bf16.
  - a tiny dummy AllGather triggers at t~0 so the one-time ncfw setup
    barrier (~36us) runs concurrent with the input streams.
  - z = h@W2 is scaled by 1/16 and cast to fp8 for a small AllGather
    (40 KB in / 320 KB out); the x16 rides the layer-2 epilogue
    activation (out = 16*psum + b2). Layer-2 accumulates two adj chunk
    pairs concurrently via PE column groups (tile_position 0 / 64).
  - log_softmax tail identical to the baseline (PE transposes of
    logits^T chunks + DVE/ACT reductions), EXP/LN tables preloaded.

kernel(**inputs) takes FULL inputs and returns the FULL [N, C] output.
"""

import numpy as np
import ml_dtypes

import concourse.bass as bass
import concourse.mybir as mybir
import concourse.tile as tile
from concourse import bacc
from concourse.bass_utils import run_bass_kernel_spmd
from concourse.masks import make_identity

NCORES = 8
N_FULL = 8192
NFEAT = 512
NHID = 128
NCLASS = 40
F32 = mybir.dt.float32
BF16 = mybir.dt.bfloat16
FP8 = mybir.dt.float8e4
ZSCALE = 16.0


def build(n_total: int = N_FULL):
    """Build the SPMD Bass graph for one core (same program on all 8)."""
    M = n_total // NCORES          # rows owned by this core
    K = n_total // 128             # 128-row contraction chunks (64)
    K2 = K // 2                    # fp8 DoubleRow chunk pairs (32)
    G = n_total // 512             # 512-node S^T column groups (16)
    MC = M // 128                  # 128-row output chunks on this core (8)
    A = M // 128                   # local 128-node chunks (8)
    DF = NFEAT // 128              # feature chunks (4)
    MW = min(512, M)               # free-dim split width
    MH = M // MW                   # halves of the local rows (2)

    nc = bacc.Bacc(
        "TRN2", target_bir_lowering=False, debug=False,
        enable_asserts=True, num_devices=NCORES,
    )

    # xb[p, g*DF*512 + d*512 + m'] = x[g*512+m', d*128+p]   (replicated)
    xb = nc.dram_tensor("xb", [128, G * DF * 512], BF16, kind="ExternalInput")
    # adjp[p, ((k2*2+i)*M) + m] = adj[(k2*2+i)*128+p, c0+m]  fp8
    adjp = nc.dram_tensor("adjp", [128, K * M], FP8, kind="ExternalInput")
    w1 = nc.dram_tensor("w1", [NFEAT, NHID], F32, kind="ExternalInput")
    b1 = nc.dram_tensor("b1", [NHID, 1], F32, kind="ExternalInput")
    w2 = nc.dram_tensor("w2", [NHID, NCLASS], F32, kind="ExternalInput")
    b2 = nc.dram_tensor("b2", [NCLASS, 1], F32, kind="ExternalInput")
    # out[p, a*NCLASS + c] = log_softmax of row a*128+p; host re-interleaves.
    out_ext = nc.dram_tensor("out", [128, MC * NCLASS], F32, kind="ExternalOutput")

    rg = [list(range(NCORES))]

    with tile.TileContext(nc) as tc:
        with (
            tc.tile_pool(name="resident", bufs=1) as res,
            tc.tile_pool(name="dram", bufs=1, space="DRAM") as dram,
        ):
            adjres = res.tile([128, K2, 2, M], FP8)        # adj_t^T resident
            xst = res.tile([128, G, DF, 512], BF16)        # replicated x
            sTt = res.tile([128, n_total], BF16)           # S^T = (x@W1)^T
            sres = res.tile([128, K, NHID], BF16)          # S chunks [node,k,hid]
            hTt = res.tile([128, M], BF16)                 # layer-1 out, [hid, m]
            zres = res.tile([128, K2, 2, NCLASS], FP8)     # gathered z/16, fp8
            zloc = res.tile([128, A, NCLASS], FP8)         # local z/16, fp8
            w1bf = res.tile([128, DF, NHID], BF16)
            w2bf = res.tile([128, NCLASS], BF16)
            b1sb = res.tile([128, 1], F32)
            b2sb = res.tile([NCLASS, 1], F32)
            ident = res.tile([128, 128], F32)
            lTsb = res.tile([NCLASS, M], F32)              # logits^T (+b2)
            osb = res.tile([128, MC, NCLASS], F32)         # final log-softmax out
            scr = res.tile([128, 1], F32)                  # act-table warmup

            # collective bounce buffers (internal DRAM).
            z_in = dram.tile([128, A * NCLASS], FP8)
            z_out = dram.tile([NCORES * 128, A * NCLASS], FP8, addr_space="Shared")

            make_identity(nc, ident[:, :])

            # ---- consts on the gpsimd SWDGE queue ----
            with tc.tile_pool(name="consts", bufs=1) as cst:
                w1st = cst.tile([128, DF, NHID], F32)
                w2st = cst.tile([128, NCLASS], F32)
                nc.gpsimd.dma_start(
                    out=w1st[:, :, :],
                    in_=w1.ap().rearrange("(a p) f -> p a f", p=128),
                )
                nc.gpsimd.tensor_copy(w1bf[:, :, :], w1st[:, :, :])
                nc.gpsimd.dma_start(out=w2st[:, :], in_=w2.ap())
                nc.gpsimd.tensor_copy(w2bf[:, :], w2st[:, :])
                nc.gpsimd.dma_start(out=b1sb[:, :], in_=b1.ap())
                nc.gpsimd.dma_start(out=b2sb[:, :], in_=b2.ap())

                # EXP/LN activation tables into their slots before the tail.
                nc.scalar.activation(
                    scr[:, :], b1sb[:, :], mybir.ActivationFunctionType.Exp
                )
                nc.scalar.activation(
                    scr[:, :], scr[:, :], mybir.ActivationFunctionType.Ln
                )

                # ---- early adj chunks on the SWDGE queue (layer-1 head) ----
                NG_ADJ = 8
                adjv = adjp.ap().rearrange("p (k i m) -> p k i m", k=K2, i=2)
                for k2 in range(NG_ADJ):
                    nc.gpsimd.dma_start(
                        out=adjres[:, k2, :, :], in_=adjv[:, k2, :, :]
                    )

                # ---- x stream head: g0..g3, alternating rings ----
                xv = xb.ap().rearrange("p (g d m) -> p g d m", g=G, d=DF)
                for g in range(0, 4):
                    eng = nc.sync if g % 2 == 0 else nc.scalar
                    eng.dma_start(out=xst[:, g, :, :], in_=xv[:, g, :, :])

                # ---- S^T phase interleaved with layer-1 (2-group lag so the
                # xbar transposes stay ahead of the PE), the x stream tail,
                # and the ring adj chunks. Per-ring queue order is chosen so
                # transposes never head-of-line block a load the PE needs
                # sooner. ----
                sres3 = sres.rearrange("p k f -> p (k f)")
                LAG = 2
                with (
                    tc.tile_pool(name="spsum", bufs=2, space="PSUM") as spsum,
                    tc.tile_pool(name="hpsum", bufs=1, space="PSUM") as hp,
                ):
                    ph = [hp.tile([128, MW], F32, name=f"ph{m}") for m in range(MH)]

                    def l1_chunk(k):
                        k2, i = divmod(k, 2)
                        for mh in range(MH):
                            nc.tensor.matmul(
                                ph[mh][:, :],
                                sres[:, k, :],
                                adjres[:, k2, i, mh * MW:(mh + 1) * MW],
                                start=(k == 0), stop=(k == K - 1),
                            )

                    for g in range(G):
                        ps = spsum.tile([128, 512], F32, tag="ps")
                        for d in range(DF):
                            nc.tensor.matmul(
                                ps[:, :],
                                w1bf[:, d, :],
                                xst[:, g, d, :],
                                start=(d == 0), stop=(d == DF - 1),
                            )
                        nc.vector.tensor_copy(
                            sTt[:, g * 512:(g + 1) * 512], ps[:, :]
                        )
                        # S^T -> S chunks through the DMA transpose xbar
                        eng = nc.sync if g % 2 == 0 else nc.scalar
                        eng.dma_start(
                            out=sres3[:, g * 4 * NHID:(g + 1) * 4 * NHID]
                            .rearrange("p (j f) -> p j f", j=4),
                            in_=sTt[:, g * 512:(g + 1) * 512],
                            transpose=True,
                        )
                        if g + 4 < G:
                            eng2 = nc.sync if g % 2 == 0 else nc.scalar
                            eng2.dma_start(
                                out=xst[:, g + 4, :, :], in_=xv[:, g + 4, :, :]
                            )
                        # ring adj chunks, two per iteration, LAG iterations
                        # ahead of the layer-1 chunks that consume them
                        for k2 in (2 * g, 2 * g + 1):
                            if NG_ADJ <= k2 < K2:
                                eng3 = nc.sync if k2 % 2 == 0 else nc.scalar
                                eng3.dma_start(
                                    out=adjres[:, k2, :, :],
                                    in_=adjv[:, k2, :, :],
                                )
                        if g >= LAG:
                            for k in range(4 * (g - LAG), 4 * (g - LAG + 1)):
                                l1_chunk(k)

                    for k in range(4 * (G - LAG), K):
                        l1_chunk(k)

                    for mh in range(MH):
                        nc.scalar.activation(
                            hTt[:, mh * MW:(mh + 1) * MW], ph[mh][:, :],
                            mybir.ActivationFunctionType.Relu,
                            bias=b1sb[:, 0:1], scale=1.0,
                        )

            # ---- z_t = (h_t @ W2)/16 as fp8, AllGather ----
            with tc.tile_pool(name="zpsum", bufs=2, space="PSUM") as zp:
                for a in range(A):
                    pz = zp.tile([128, NCLASS], F32, tag="pz")
                    nc.tensor.matmul(
                        pz[:, :],
                        hTt[:, a * 128:(a + 1) * 128],
                        w2bf[:, :],
                        start=True, stop=True,
                    )
                    nc.scalar.activation(
                        zloc[:, a, :], pz[:, :],
                        mybir.ActivationFunctionType.Copy,
                        bias=0.0, scale=1.0 / ZSCALE,
                    )
            nc.gpsimd.dma_start(
                out=z_in.rearrange("p (a c) -> p a c", a=A), in_=zloc[:, :, :]
            )
            nc.gpsimd.collective_compute(
                "AllGather", mybir.AluOpType.bypass, replica_groups=rg,
                ins=[z_in[:, :]], outs=[z_out[:, :]],
            )

            zrf = zres.rearrange("p k i c -> p (k i) c")
            zov = z_out.rearrange("(j p) (a c) -> p j a c", p=128, a=A)
            nc.sync.dma_start(
                out=zrf[:, 0:K // 2, :].rearrange("p (j a) c -> p j a c", a=A),
                in_=zov[:, 0:NCORES // 2, :, :],
            )
            nc.scalar.dma_start(
                out=zrf[:, K // 2:, :].rearrange("p (j a) c -> p j a c", a=A),
                in_=zov[:, NCORES // 2:, :, :],
            )

            # ---- layer 2 (fp8 DoubleRow, 2 concurrent column groups) +
            # log_softmax, split per mh half so the vector/scalar softmax
            # work overlaps the other half's matmuls. ----
            with (
                tc.tile_pool(name="lpsum", bufs=1, space="PSUM") as lp,
                tc.tile_pool(name="smp", bufs=1, space="PSUM") as smp,
                tc.tile_pool(name="sms", bufs=1) as sms,
            ):
                ptrs = smp.tile([128, MC, NCLASS], F32)
                lttmp = sms.tile([NCLASS, MW], F32)
                mx = sms.tile([128, MC], F32)
                ssum = sms.tile([128, MC], F32)
                lse = sms.tile([128, MC], F32)
                bias2 = sms.tile([128, MC], F32)
                esc = sms.tile([128, MC, NCLASS], F32)
                MCH = MC // MH                 # 128-row chunks per half
                for mh in range(MH):
                    pl = lp.tile([128, MW], F32, tag="pl")
                    for k2 in range(K2):
                        # fp8 moving, two concurrent PE column groups
                        nc.tensor.matmul(
                            pl[0:NCLASS, :],
                            zres[:, k2, 0, :],
                            adjres[:, k2, 0, mh * MW:(mh + 1) * MW],
                            start=(k2 == 0), stop=(k2 == K2 - 1),
                            tile_position=(0, 0),
                            skip_group_check=True,
                        )
                        nc.tensor.matmul(
                            pl[64:64 + NCLASS, :],
                            zres[:, k2, 1, :],
                            adjres[:, k2, 1, mh * MW:(mh + 1) * MW],
                            start=(k2 == 0), stop=(k2 == K2 - 1),
                            tile_position=(0, 64),
                            skip_group_check=True,
                        )
                    nc.scalar.activation(
                        lTsb[:, mh * MW:(mh + 1) * MW], pl[64:64 + NCLASS, :],
                        mybir.ActivationFunctionType.Identity,
                        bias=b2sb[:, 0:1], scale=ZSCALE,
                    )
                    nc.scalar.activation(
                        lttmp[:, :], pl[0:NCLASS, :],
                        mybir.ActivationFunctionType.Copy,
                        bias=0.0, scale=ZSCALE,
                    )
                    nc.vector.tensor_tensor(
                        lTsb[:, mh * MW:(mh + 1) * MW],
                        lTsb[:, mh * MW:(mh + 1) * MW], lttmp[:, :],
                        op=mybir.AluOpType.add,
                    )
                    for mc in range(mh * MCH, (mh + 1) * MCH):
                        nc.tensor.transpose(
                            ptrs[:, mc, :], lTsb[:, mc * 128:(mc + 1) * 128],
                            ident[0:NCLASS, 0:NCLASS],
                        )
                        nc.vector.tensor_reduce(
                            mx[:, mc:mc + 1], ptrs[:, mc, :],
                            axis=mybir.AxisListType.X,
                            op=mybir.AluOpType.max, negate=True,
                        )
                        nc.scalar.activation(
                            esc[:, mc, :], ptrs[:, mc, :],
                            mybir.ActivationFunctionType.Exp,
                            bias=mx[:, mc:mc + 1], scale=1.0,
                            accum_out=ssum[:, mc:mc + 1],
                        )
                nc.scalar.activation(
                    lse[:, :], ssum[:, :], mybir.ActivationFunctionType.Ln,
                )
                nc.vector.tensor_sub(bias2[:, :], mx[:, :], lse[:, :])
                for mc in range(MC):
                    nc.scalar.activation(
                        osb[:, mc, :], ptrs[:, mc, :],
                        mybir.ActivationFunctionType.Identity,
                        bias=bias2[:, mc:mc + 1], scale=1.0,
                    )
            # contiguous per-partition lines; host reorders. Split in two so
            # the first half overlaps the second half's epilogue.
            oview = out_ext.ap().rearrange("p (a c) -> p a c", a=MC)
            nc.sync.dma_start(
                out=oview[:, 0:MC // 2, :], in_=osb[:, 0:MC // 2, :]
            )
            nc.sync.dma_start(
                out=oview[:, MC // 2:, :], in_=osb[:, MC // 2:, :]
            )

    nc.compile()
    return nc


_NC_CACHE = {}


def _get_nc(n_total: int):
    if n_total not in _NC_CACHE:
        _NC_CACHE[n_total] = build(n_total)
    return _NC_CACHE[n_total]


def make_in_maps(x, adj, W1, b1, W2, b2):
    n_total = x.shape[0]
    m = n_total // NCORES
    g = n_total // 512
    k2 = n_total // 256
    # xb[p, g, d, m'] = x^T[d*128+p, g*512+m']  (replicated, bf16)
    xT = np.ascontiguousarray(x.T.astype(ml_dtypes.bfloat16))
    xbp = np.ascontiguousarray(
        xT.reshape(DFG := NFEAT // 128, 128, g, 512).transpose(1, 2, 0, 3)
    ).reshape(128, g * DFG * 512)
    in_maps = []
    for t in range(NCORES):
        c0 = t * m
        # adjp[p, k2, i, m] = adj[(k2*2+i)*128+p, c0+m]  fp8
        asl = adj[:, c0:c0 + m].astype(ml_dtypes.float8_e4m3)
        ap8 = np.ascontiguousarray(
            asl.reshape(k2, 2, 128, m).transpose(2, 0, 1, 3)
        ).reshape(128, k2 * 2 * m)
        in_maps.append({
            "xb": xbp,
            "adjp": ap8,
            "w1": np.ascontiguousarray(W1),
            "b1": np.ascontiguousarray(b1.reshape(NHID, 1)),
            "w2": np.ascontiguousarray(W2),
            "b2": np.ascontiguousarray(b2.reshape(NCLASS, 1)),
        })
    return in_maps


def _assemble(res_list):
    """[128, MC*NCLASS] per core -> [N, NCLASS]."""
    outs = []
    for r in res_list:
        o = np.asarray(r["out"])
        mc = o.shape[1] // NCLASS
        outs.append(
            o.reshape(128, mc, NCLASS).transpose(1, 0, 2).reshape(-1, NCLASS)
        )
    return np.concatenate(outs, axis=0)


def kernel(x, adj, W1, b1, W2, b2):
    x = np.asarray(x, dtype=np.float32)
    adj = np.asarray(adj, dtype=np.float32)
    W1 = np.asarray(W1, dtype=np.float32)
    b1 = np.asarray(b1, dtype=np.float32)
    W2 = np.asarray(W2, dtype=np.float32)
    b2 = np.asarray(b2, dtype=np.float32)
    nc = _get_nc(x.shape[0])
    in_maps = make_in_maps(x, adj, W1, b1, W2, b2)
    res = run_bass_kernel_spmd(nc, in_maps, list(range(NCORES)))
    return _assemble([res.results[i] for i in range(NCORES)])


# revision 18
# speedup vs baseline: 1.0550x; 1.0550x over previous
"""Trainium2 Bass kernel for a 2-layer GCN (gnn_message_passing).

Reference computation (all f32 inputs):
    h      = relu(adj @ (x @ W1) + b1)        adj: [N, N], x: [N, F]
    logits = adj @ (h @ W2) + b2
    out    = log_softmax(logits, axis=1)       out: [N, C]

Distribution: 1-D row partition over 8 NeuronCores. Core t owns rows
R_t = [t*M, (t+1)*M). adj is symmetric, so adj[:, R_t] (shipped in
natural column-slice layout) doubles as the transposed moving operand
for both layers.

v3 design (vs the f32/bf16 + double-AllGather baseline):
  - adj ships from the HOST as fp8e4m3 in a paired-chunk layout
    [128, K2, 2, M] (8 MB/core instead of 32): DMA lands directly in
    the resident SBUF tile - no staging pools, no on-chip casts. The
    fp8 pairing feeds layer-2's DoubleRow matmuls (0.5 cyc/row) and
    layer-1 streams it as a plain fp8 moving operand (1 cyc/row).
  - the S AllGather is gone: every core computes the FULL S^T = W1^T x
    locally from a replicated bf16 x (fat 512-wide matmuls, 4 ldw per
    512 columns), then converts S^T -> S chunks with DMA-xbar
    transposes (off the PE). S-in-fp8 fails the accuracy budget
    (2.5e-2 > 2e-2 measured), so S stays bf16 and layer-1 runs mixed
    fp8 x bf16.
  - a tiny dummy AllGather triggers at t~0 so the one-time ncfw setup
    barrier (~36us) runs concurrent with the input streams.
  - z = h@W2 is scaled by 1/16 and cast to fp8 for a small AllGather
    (40 KB in / 320 KB out); the x16 rides the layer-2 epilogue
    activation (out = 16*psum + b2). Layer-2 accumulates two adj chunk
    pairs concurrently via PE column groups (tile_position 0 / 64).
  - log_softmax tail identical to the baseline (PE transposes of
    logits^T chunks + DVE/ACT reductions), EXP/LN tables preloaded.

kernel(**inputs) takes FULL inputs and returns the FULL [N, C] output.
"""

import numpy as np
import ml_dtypes

import concourse.bass as bass
import concourse.mybir as mybir
import concourse.tile as tile
from concourse import bacc
from concourse.bass_utils import run_bass_kernel_spmd
from concourse.masks import make_identity

NCORES = 8
N_FULL = 8192
NFEAT = 512
NHID = 128
NCLASS = 40
F32 = mybir.dt.float32
BF16 = mybir.dt.bfloat16
FP8 = mybir.dt.float8e4
ZSCALE = 16.0


def build(n_total: int = N_FULL):
    """Build the SPMD Bass graph for one core (same program on all 8)."""
    M = n_total // NCORES          # rows owned by this core
    K = n_total // 128             # 128-row contraction chunks (64)
    K2 = K // 2                    # fp8 DoubleRow chunk pairs (32)
    G = n_total // 512             # 512-node S^T column groups (16)
    MC = M // 128                  # 128-row output chunks on this core (8)
    A = M // 128                   # local 128-node chunks (8)
    DF = NFEAT // 128              # feature chunks (4)
    MW = min(512, M)               # free-dim split width
    MH = M // MW                   # halves of the local rows (2)

    nc = bacc.Bacc(
        "TRN2", target_bir_lowering=False, debug=False,
        enable_asserts=True, num_devices=NCORES,
    )

    # xb[p, g*DF*512 + d*512 + m'] = x[g*512+m', d*128+p]   (replicated)
    xb = nc.dram_tensor("xb", [128, G * DF * 512], BF16, kind="ExternalInput")
    # adjp[p, ((k2*2+i)*M) + m] = adj[(k2*2+i)*128+p, c0+m]  fp8
    adjp = nc.dram_tensor("adjp", [128, K * M], FP8, kind="ExternalInput")
    w1 = nc.dram_tensor("w1", [NFEAT, NHID], F32, kind="ExternalInput")
    b1 = nc.dram_tensor("b1", [NHID, 1], F32, kind="ExternalInput")
    w2 = nc.dram_tensor("w2", [NHID, NCLASS], F32, kind="ExternalInput")
    b2 = nc.dram_tensor("b2", [NCLASS, 1], F32, kind="ExternalInput")
    # out[p, a*NCLASS + c] = log_softmax of row a*128+p; host re-interleaves.
    out_ext = nc.dram_tensor("out", [128, MC * NCLASS], F32, kind="ExternalOutput")

    rg = [list(range(NCORES))]

    with tile.TileContext(nc) as tc:
        with (
            tc.tile_pool(name="resident", bufs=1) as res,
            tc.tile_pool(name="dram", bufs=1, space="DRAM") as dram,
        ):
            adjres = res.tile([128, K2, 2, M], FP8)        # adj_t^T resident
            xst = res.tile([128, G, DF, 512], BF16)        # replicated x
            sTt = res.tile([128, n_total], BF16)           # S^T = (x@W1)^T
            sres = res.tile([128, K, NHID], BF16)          # S chunks [node,k,hid]
            hTt = res.tile([128, M], BF16)                 # layer-1 out, [hid, m]
            zres = res.tile([128, K2, 2, NCLASS], FP8)     # gathered z/16, fp8
            zloc = res.tile([128, A, NCLASS], FP8)         # local z/16, fp8
            w1bf = res.tile([128, DF, NHID], BF16)
            w2bf = res.tile([128, NCLASS], BF16)
            b1sb = res.tile([128, 1], F32)
            b2sb = res.tile([NCLASS, 1], F32)
            ident = res.tile([128, 128], F32)
            lTsb = res.tile([NCLASS, M], F32)              # logits^T (+b2)
            osb = res.tile([128, MC, NCLASS], F32)         # final log-softmax out
            scr = res.tile([128, 1], F32)                  # act-table warmup
            dum = res.tile([1, 16], BF16)                  # dummy-AG payload

            # collective bounce buffers (internal DRAM).
            d_in = dram.tile([1, 16], BF16)
            d_out = dram.tile([NCORES, 16], BF16, addr_space="Shared")
            z_in = dram.tile([128, A * NCLASS], FP8)
            z_out = dram.tile([NCORES * 128, A * NCLASS], FP8, addr_space="Shared")

            # ---- dummy AllGather, triggered first on the gpsimd queue. The
            # ncfw setup barrier runs ~[18, 60]us regardless; the first
            # gather additionally starves the HWDGE rings for ~25us while it
            # executes (right after setup), and runs a one-time slow path.
            # Spending that on a 32-byte dummy keeps both costs off the real
            # z AllGather, which then runs warm in ~7us. The starvation
            # window lands at ~[60, 88]us - after the input streams are
            # done. ----
            nc.gpsimd.memset(dum[:, :], 0.0)
            nc.gpsimd.dma_start(out=d_in[:, :], in_=dum[:, :])
            nc.gpsimd.collective_compute(
                "AllGather", mybir.AluOpType.bypass, replica_groups=rg,
                ins=[d_in[:, :]], outs=[d_out[:, :]],
            )

            make_identity(nc, ident[:, :])

            # ---- consts on the gpsimd SWDGE queue ----
            with tc.tile_pool(name="consts", bufs=1) as cst:
                w1st = cst.tile([128, DF, NHID], F32)
                w2st = cst.tile([128, NCLASS], F32)
                nc.gpsimd.dma_start(
                    out=w1st[:, :, :],
                    in_=w1.ap().rearrange("(a p) f -> p a f", p=128),
                )
                nc.gpsimd.tensor_copy(w1bf[:, :, :], w1st[:, :, :])
                nc.gpsimd.dma_start(out=w2st[:, :], in_=w2.ap())
                nc.gpsimd.tensor_copy(w2bf[:, :], w2st[:, :])
                nc.gpsimd.dma_start(out=b1sb[:, :], in_=b1.ap())
                nc.gpsimd.dma_start(out=b2sb[:, :], in_=b2.ap())

                # EXP/LN activation tables into their slots before the tail.
                nc.scalar.activation(
                    scr[:, :], b1sb[:, :], mybir.ActivationFunctionType.Exp
                )
                nc.scalar.activation(
                    scr[:, :], scr[:, :], mybir.ActivationFunctionType.Ln
                )

                # ---- early adj chunks on the SWDGE queue (layer-1 head) ----
                NG_ADJ = 8
                adjv = adjp.ap().rearrange("p (k i m) -> p k i m", k=K2, i=2)
                for k2 in range(NG_ADJ):
                    nc.gpsimd.dma_start(
                        out=adjres[:, k2, :, :], in_=adjv[:, k2, :, :]
                    )

                # ---- x stream: all 16 groups on the sync ring, in order.
                # One ring sustains ~180 GB/s -> ~2.8us per 512KB group,
                # which paces the S^T phase. ----
                xv = xb.ap().rearrange("p (g d m) -> p g d m", g=G, d=DF)
                for g in range(G):
                    nc.sync.dma_start(out=xst[:, g, :, :], in_=xv[:, g, :, :])

                # ---- S^T phase interleaved with layer-1 (2-group lag so the
                # xbar transposes stay ahead of the PE). The scalar ring
                # carries ONLY the transposes + later adj chunks, so a
                # transpose waiting on the PE never delays a load the PE
                # needs sooner. ----
                sres3 = sres.rearrange("p k f -> p (k f)")
                LAG = 2
                with (
                    tc.tile_pool(name="spsum", bufs=2, space="PSUM") as spsum,
                    tc.tile_pool(name="hpsum", bufs=1, space="PSUM") as hp,
                ):
                    ph = [hp.tile([128, MW], F32, name=f"ph{m}") for m in range(MH)]

                    def l1_chunk(k):
                        k2, i = divmod(k, 2)
                        for mh in range(MH):
                            nc.tensor.matmul(
                                ph[mh][:, :],
                                sres[:, k, :],
                                adjres[:, k2, i, mh * MW:(mh + 1) * MW],
                                start=(k == 0), stop=(k == K - 1),
                            )

                    for g in range(G):
                        ps = spsum.tile([128, 512], F32, tag="ps")
                        for d in range(DF):
                            nc.tensor.matmul(
                                ps[:, :],
                                w1bf[:, d, :],
                                xst[:, g, d, :],
                                start=(d == 0), stop=(d == DF - 1),
                            )
                        nc.vector.tensor_copy(
                            sTt[:, g * 512:(g + 1) * 512], ps[:, :]
                        )
                        # S^T -> S chunks through the DMA transpose xbar
                        nc.scalar.dma_start(
                            out=sres3[:, g * 4 * NHID:(g + 1) * 4 * NHID]
                            .rearrange("p (j f) -> p j f", j=4),
                            in_=sTt[:, g * 512:(g + 1) * 512],
                            transpose=True,
                        )
                        # ring adj chunks, two per iteration, LAG iterations
                        # ahead of the layer-1 chunks that consume them
                        for k2 in (2 * g, 2 * g + 1):
                            if NG_ADJ <= k2 < K2:
                                nc.scalar.dma_start(
                                    out=adjres[:, k2, :, :],
                                    in_=adjv[:, k2, :, :],
                                )
                        if g >= LAG:
                            for k in range(4 * (g - LAG), 4 * (g - LAG + 1)):
                                l1_chunk(k)

                    for k in range(4 * (G - LAG), K):
                        l1_chunk(k)

                    for mh in range(MH):
                        nc.scalar.activation(
                            hTt[:, mh * MW:(mh + 1) * MW], ph[mh][:, :],
                            mybir.ActivationFunctionType.Relu,
                            bias=b1sb[:, 0:1], scale=1.0,
                        )

            # ---- z_t = (h_t @ W2)/16 as fp8, AllGather ----
            with tc.tile_pool(name="zpsum", bufs=2, space="PSUM") as zp:
                for a in range(A):
                    pz = zp.tile([128, NCLASS], F32, tag="pz")
                    nc.tensor.matmul(
                        pz[:, :],
                        hTt[:, a * 128:(a + 1) * 128],
                        w2bf[:, :],
                        start=True, stop=True,
                    )
                    nc.scalar.activation(
                        zloc[:, a, :], pz[:, :],
                        mybir.ActivationFunctionType.Copy,
                        bias=0.0, scale=1.0 / ZSCALE,
                    )
            nc.gpsimd.dma_start(
                out=z_in.rearrange("p (a c) -> p a c", a=A), in_=zloc[:, :, :]
            )
            nc.gpsimd.collective_compute(
                "AllGather", mybir.AluOpType.bypass, replica_groups=rg,
                ins=[z_in[:, :]], outs=[z_out[:, :]],
            )

            zrf = zres.rearrange("p k i c -> p (k i) c")
            zov = z_out.rearrange("(j p) (a c) -> p j a c", p=128, a=A)
            nc.sync.dma_start(
                out=zrf[:, 0:K // 2, :].rearrange("p (j a) c -> p j a c", a=A),
                in_=zov[:, 0:NCORES // 2, :, :],
            )
            nc.scalar.dma_start(
                out=zrf[:, K // 2:, :].rearrange("p (j a) c -> p j a c", a=A),
                in_=zov[:, NCORES // 2:, :, :],
            )

            # ---- layer 2 (fp8 DoubleRow, 2 concurrent column groups) +
            # log_softmax, split per mh half so the vector/scalar softmax
            # work overlaps the other half's matmuls. ----
            with (
                tc.tile_pool(name="lpsum", bufs=1, space="PSUM") as lp,
                tc.tile_pool(name="smp", bufs=1, space="PSUM") as smp,
                tc.tile_pool(name="sms", bufs=1) as sms,
            ):
                ptrs = smp.tile([128, MC, NCLASS], F32)
                lttmp = sms.tile([NCLASS, MW], F32)
                mx = sms.tile([128, MC], F32)
                ssum = sms.tile([128, MC], F32)
                lse = sms.tile([128, MC], F32)
                bias2 = sms.tile([128, MC], F32)
                esc = sms.tile([128, MC, NCLASS], F32)
                MCH = MC // MH                 # 128-row chunks per half
                for mh in range(MH):
                    pl = lp.tile([128, MW], F32, tag="pl")
                    for k2 in range(K2):
                        # fp8 moving, two concurrent PE column groups
                        nc.tensor.matmul(
                            pl[0:NCLASS, :],
                            zres[:, k2, 0, :],
                            adjres[:, k2, 0, mh * MW:(mh + 1) * MW],
                            start=(k2 == 0), stop=(k2 == K2 - 1),
                            tile_position=(0, 0),
                            skip_group_check=True,
                        )
                        nc.tensor.matmul(
                            pl[64:64 + NCLASS, :],
                            zres[:, k2, 1, :],
                            adjres[:, k2, 1, mh * MW:(mh + 1) * MW],
                            start=(k2 == 0), stop=(k2 == K2 - 1),
                            tile_position=(0, 64),
                            skip_group_check=True,
                        )
                    nc.scalar.activation(
                        lTsb[:, mh * MW:(mh + 1) * MW], pl[64:64 + NCLASS, :],
                        mybir.ActivationFunctionType.Identity,
                        bias=b2sb[:, 0:1], scale=ZSCALE,
                    )
                    nc.scalar.activation(
                        lttmp[:, :], pl[0:NCLASS, :],
                        mybir.ActivationFunctionType.Copy,
                        bias=0.0, scale=ZSCALE,
                    )
                    nc.vector.tensor_tensor(
                        lTsb[:, mh * MW:(mh + 1) * MW],
                        lTsb[:, mh * MW:(mh + 1) * MW], lttmp[:, :],
                        op=mybir.AluOpType.add,
                    )
                    for mc in range(mh * MCH, (mh + 1) * MCH):
                        nc.tensor.transpose(
                            ptrs[:, mc, :], lTsb[:, mc * 128:(mc + 1) * 128],
                            ident[0:NCLASS, 0:NCLASS],
                        )
                        nc.vector.tensor_reduce(
                            mx[:, mc:mc + 1], ptrs[:, mc, :],
                            axis=mybir.AxisListType.X,
                            op=mybir.AluOpType.max, negate=True,
                        )
                        nc.scalar.activation(
                            esc[:, mc, :], ptrs[:, mc, :],
                            mybir.ActivationFunctionType.Exp,
                            bias=mx[:, mc:mc + 1], scale=1.0,
                            accum_out=ssum[:, mc:mc + 1],
                        )
                nc.scalar.activation(
                    lse[:, :], ssum[:, :], mybir.ActivationFunctionType.Ln,
                )
                nc.vector.tensor_sub(bias2[:, :], mx[:, :], lse[:, :])
                for mc in range(MC):
                    nc.scalar.activation(
                        osb[:, mc, :], ptrs[:, mc, :],
                        mybir.ActivationFunctionType.Identity,
                        bias=bias2[:, mc:mc + 1], scale=1.0,
                    )
            # contiguous per-partition lines; host reorders. Split in two so
            # the first half overlaps the second half's epilogue.
            oview = out_ext.ap().rearrange("p (a c) -> p a c", a=MC)
            nc.sync.dma_start(
                out=oview[:, 0:MC // 2, :], in_=osb[:, 0:MC // 2, :]
            )
            nc.sync.dma_start(
                out=oview[:, MC // 2:, :], in_=osb[:, MC // 2:, :]
            )

    nc.compile()
    return nc


_NC_CACHE = {}


def _get_nc(n_total: int):
    if n_total not in _NC_CACHE:
        _NC_CACHE[n_total] = build(n_total)
    return _NC_CACHE[n_total]


def make_in_maps(x, adj, W1, b1, W2, b2):
    n_total = x.shape[0]
    m = n_total // NCORES
    g = n_total // 512
    k2 = n_total // 256
    # xb[p, g, d, m'] = x^T[d*128+p, g*512+m']  (replicated, bf16)
    xT = np.ascontiguousarray(x.T.astype(ml_dtypes.bfloat16))
    xbp = np.ascontiguousarray(
        xT.reshape(DFG := NFEAT // 128, 128, g, 512).transpose(1, 2, 0, 3)
    ).reshape(128, g * DFG * 512)
    in_maps = []
    for t in range(NCORES):
        c0 = t * m
        # adjp[p, k2, i, m] = adj[(k2*2+i)*128+p, c0+m]  fp8
        asl = adj[:, c0:c0 + m].astype(ml_dtypes.float8_e4m3)
        ap8 = np.ascontiguousarray(
            asl.reshape(k2, 2, 128, m).transpose(2, 0, 1, 3)
        ).reshape(128, k2 * 2 * m)
        in_maps.append({
            "xb": xbp,
            "adjp": ap8,
            "w1": np.ascontiguousarray(W1),
            "b1": np.ascontiguousarray(b1.reshape(NHID, 1)),
            "w2": np.ascontiguousarray(W2),
            "b2": np.ascontiguousarray(b2.reshape(NCLASS, 1)),
        })
    return in_maps


def _assemble(res_list):
    """[128, MC*NCLASS] per core -> [N, NCLASS]."""
    outs = []
    for r in res_list:
        o = np.asarray(r["out"])
        mc = o.shape[1] // NCLASS
        outs.append(
            o.reshape(128, mc, NCLASS).transpose(1, 0, 2).reshape(-1, NCLASS)
        )
    return np.concatenate(outs, axis=0)


def kernel(x, adj, W1, b1, W2, b2):
    x = np.asarray(x, dtype=np.float32)
    adj = np.asarray(adj, dtype=np.float32)
    W1 = np.asarray(W1, dtype=np.float32)
    b1 = np.asarray(b1, dtype=np.float32)
    W2 = np.asarray(W2, dtype=np.float32)
    b2 = np.asarray(b2, dtype=np.float32)
    nc = _get_nc(x.shape[0])
    in_maps = make_in_maps(x, adj, W1, b1, W2, b2)
    res = run_bass_kernel_spmd(nc, in_maps, list(range(NCORES)))
    return _assemble([res.results[i] for i in range(NCORES)])


# revision 24
# speedup vs baseline: 1.4410x; 1.3659x over previous
"""Trainium2 Bass kernel for a 2-layer GCN (gnn_message_passing).

Reference computation (all f32 inputs):
    h      = relu(adj @ (x @ W1) + b1)        adj: [N, N], x: [N, F]
    logits = adj @ (h @ W2) + b2
    out    = log_softmax(logits, axis=1)       out: [N, C]

Distribution: 1-D row partition over 8 NeuronCores. Core t owns rows
R_t = [t*M, (t+1)*M). adj is symmetric, so adj[:, R_t] (shipped in
natural column-slice layout) doubles as the transposed moving operand
for both layers.

v3 design (vs the f32/bf16 + double-AllGather baseline):
  - adj ships from the HOST as fp8e4m3 in a paired-chunk layout
    [128, K2, 2, M] (8 MB/core instead of 32): DMA lands directly in
    the resident SBUF tile - no staging pools, no on-chip casts. The
    fp8 pairing feeds layer-2's DoubleRow matmuls (0.5 cyc/row) and
    layer-1 streams it as a plain fp8 moving operand (1 cyc/row).
  - the S AllGather is gone: every core computes the FULL S^T = W1^T x
    locally from a replicated bf16 x (fat 512-wide matmuls, 4 ldw per
    512 columns), then converts S^T -> S chunks with DMA-xbar
    transposes (off the PE). S-in-fp8 fails the accuracy budget
    (2.5e-2 > 2e-2 measured), so S stays bf16 and layer-1 runs mixed
    fp8 x bf16.
  - a tiny dummy AllGather triggers at t~0 so the one-time ncfw setup
    barrier (~36us) runs concurrent with the input streams.
  - z = h@W2 is scaled by 1/16 and cast to fp8 for a small AllGather
    (40 KB in / 320 KB out); the x16 rides the layer-2 epilogue
    activation (out = 16*psum + b2). Layer-2 accumulates two adj chunk
    pairs concurrently via PE column groups (tile_position 0 / 64).
  - log_softmax tail identical to the baseline (PE transposes of
    logits^T chunks + DVE/ACT reductions), EXP/LN tables preloaded.

kernel(**inputs) takes FULL inputs and returns the FULL [N, C] output.
"""

import numpy as np
import ml_dtypes

import concourse.bass as bass
import concourse.mybir as mybir
import concourse.tile as tile
from concourse import bacc
from concourse.bass_utils import run_bass_kernel_spmd
from concourse.masks import make_identity

NCORES = 8
N_FULL = 8192
NFEAT = 512
NHID = 128
NCLASS = 40
F32 = mybir.dt.float32
BF16 = mybir.dt.bfloat16
FP8 = mybir.dt.float8e4
ZSCALE = 16.0


def build(n_total: int = N_FULL):
    """Build the SPMD Bass graph for one core (same program on all 8)."""
    M = n_total // NCORES          # rows owned by this core
    K = n_total // 128             # 128-row contraction chunks (64)
    K2 = K // 2                    # fp8 DoubleRow chunk pairs (32)
    G = n_total // 512             # 512-node S^T column groups (16)
    MC = M // 128                  # 128-row output chunks on this core (8)
    A = M // 128                   # local 128-node chunks (8)
    DF = NFEAT // 128              # feature chunks (4)
    MW = min(512, M)               # free-dim split width
    MH = M // MW                   # halves of the local rows (2)

    nc = bacc.Bacc(
        "TRN2", target_bir_lowering=False, debug=False,
        enable_asserts=True, num_devices=NCORES,
    )

    # xb[p, g*DF*512 + d*512 + m'] = x[g*512+m', d*128+p]   (replicated)
    xb = nc.dram_tensor("xb", [128, G * DF * 512], BF16, kind="ExternalInput")
    # adjp[p, ((k2*2+i)*M) + m] = adj[(k2*2+i)*128+p, c0+m]  fp8
    adjp = nc.dram_tensor("adjp", [128, K * M], FP8, kind="ExternalInput")
    w1 = nc.dram_tensor("w1", [NFEAT, NHID], F32, kind="ExternalInput")
    b1 = nc.dram_tensor("b1", [NHID, 1], F32, kind="ExternalInput")
    w2 = nc.dram_tensor("w2", [NHID, NCLASS], F32, kind="ExternalInput")
    b2 = nc.dram_tensor("b2", [NCLASS, 1], F32, kind="ExternalInput")
    # out[p, a*NCLASS + c] = log_softmax of row a*128+p; host re-interleaves.
    out_ext = nc.dram_tensor("out", [128, MC * NCLASS], F32, kind="ExternalOutput")

    rg = [list(range(NCORES))]

    with tile.TileContext(nc) as tc:
        with (
            tc.tile_pool(name="resident", bufs=1) as res,
            tc.tile_pool(name="dram", bufs=1, space="DRAM") as dram,
        ):
            adjres = res.tile([128, K2, 2, M], FP8)        # adj_t^T resident
            xst = res.tile([128, G, DF, 512], BF16)        # replicated x
            sTt = res.tile([128, n_total], BF16)           # S^T = (x@W1)^T
            sres = res.tile([128, K, NHID], BF16)          # S chunks [node,k,hid]
            hTt = res.tile([128, M], BF16)                 # layer-1 out, [hid, m]
            zres = res.tile([128, K2, 2, NCLASS], FP8)     # gathered z/16, fp8
            zloc = res.tile([128, A, NCLASS], FP8)         # local z/16, fp8
            w1bf = res.tile([128, DF, NHID], BF16)
            w2bf = res.tile([128, NCLASS], BF16)
            identb = res.tile([128, 128], BF16)
            b1sb = res.tile([128, 1], F32)
            b2sb = res.tile([NCLASS, 1], F32)
            ident = res.tile([128, 128], F32)
            lTsb = res.tile([NCLASS, M], F32)              # logits^T (+b2)
            osb = res.tile([128, MC, NCLASS], F32)         # final log-softmax out
            scr = res.tile([128, 1], F32)                  # act-table warmup
            dum = res.tile([1, 16], BF16)                  # dummy-AG payload

            # collective bounce buffers (internal DRAM).
            d_in = dram.tile([1, 16], BF16)
            d_out = dram.tile([NCORES, 16], BF16, addr_space="Shared")
            z_in = dram.tile([128, A * NCLASS], FP8)
            z_out = dram.tile([NCORES * 128, A * NCLASS], FP8, addr_space="Shared")

            # ---- dummy AllGather, triggered first on the gpsimd queue. The
            # ncfw setup barrier runs ~[18, 60]us regardless; the first
            # gather additionally starves the HWDGE rings for ~25us while it
            # executes (right after setup), and runs a one-time slow path.
            # Spending that on a 32-byte dummy keeps both costs off the real
            # z AllGather, which then runs warm in ~7us. The starvation
            # window lands at ~[60, 88]us - after the input streams are
            # done. ----
            nc.gpsimd.memset(dum[:, :], 0.0)
            nc.gpsimd.dma_start(out=d_in[:, :], in_=dum[:, :])
            nc.gpsimd.collective_compute(
                "AllGather", mybir.AluOpType.bypass, replica_groups=rg,
                ins=[d_in[:, :]], outs=[d_out[:, :]],
            )

            make_identity(nc, ident[:, :])
            nc.gpsimd.tensor_copy(identb[:, :], ident[:, :])

            # ---- consts on the gpsimd SWDGE queue ----
            with tc.tile_pool(name="consts", bufs=1) as cst:
                w1st = cst.tile([128, DF, NHID], F32)
                w2st = cst.tile([128, NCLASS], F32)
                nc.gpsimd.dma_start(
                    out=w1st[:, :, :],
                    in_=w1.ap().rearrange("(a p) f -> p a f", p=128),
                )
                nc.gpsimd.tensor_copy(w1bf[:, :, :], w1st[:, :, :])
                nc.gpsimd.dma_start(out=w2st[:, :], in_=w2.ap())
                nc.gpsimd.tensor_copy(w2bf[:, :], w2st[:, :])
                nc.gpsimd.dma_start(out=b1sb[:, :], in_=b1.ap())
                nc.gpsimd.dma_start(out=b2sb[:, :], in_=b2.ap())

                # EXP/LN activation tables into their slots before the tail.
                nc.scalar.activation(
                    scr[:, :], b1sb[:, :], mybir.ActivationFunctionType.Exp
                )
                nc.scalar.activation(
                    scr[:, :], scr[:, :], mybir.ActivationFunctionType.Ln
                )

                # ---- early adj chunks on the SWDGE queue (layer-1 head) ----
                NG_ADJ = 8
                adjv = adjp.ap().rearrange("p (k i m) -> p k i m", k=K2, i=2)
                for k2 in range(NG_ADJ):
                    nc.gpsimd.dma_start(
                        out=adjres[:, k2, :, :], in_=adjv[:, k2, :, :]
                    )

                # ---- x stream: all 16 groups on the sync ring, in order.
                # One ring sustains ~180 GB/s -> ~2.8us per 512KB group,
                # which paces the S^T phase. ----
                xv = xb.ap().rearrange("p (g d m) -> p g d m", g=G, d=DF)
                for g in range(G):
                    nc.sync.dma_start(out=xst[:, g, :, :], in_=xv[:, g, :, :])

                # ---- remaining adj chunks on the scalar ring (pure loads,
                # no interleaved waits: the ring packs back-to-back and the
                # whole stream is resident before the ncfw collective-setup
                # window (~[60, 88]us) can starve the rings). ----
                for k2 in range(NG_ADJ, K2):
                    nc.scalar.dma_start(
                        out=adjres[:, k2, :, :], in_=adjv[:, k2, :, :]
                    )

                # ---- S^T phase interleaved with PE transposes (S^T -> S
                # chunks) and layer-1 (2-group lag). Everything on the PE,
                # in order: the schedule is deterministic and the rings stay
                # untouched. ----
                LAG = 2
                with (
                    tc.tile_pool(name="spsum", bufs=2, space="PSUM") as spsum,
                    tc.tile_pool(name="trpsum", bufs=2, space="PSUM") as trp,
                    tc.tile_pool(name="hpsum", bufs=1, space="PSUM") as hp,
                ):
                    ph = [hp.tile([128, MW], F32, name=f"ph{m}") for m in range(MH)]

                    def l1_chunk(k):
                        k2, i = divmod(k, 2)
                        for mh in range(MH):
                            nc.tensor.matmul(
                                ph[mh][:, :],
                                sres[:, k, :],
                                adjres[:, k2, i, mh * MW:(mh + 1) * MW],
                                start=(k == 0), stop=(k == K - 1),
                            )

                    def transpose_group(gt):
                        # S^T -> S chunks via PE transpose (identity moving);
                        # input is the DVE copy of group gt, issued a full
                        # group earlier, so the PE never waits on it.
                        pt = trp.tile([128, 4, NHID], BF16, tag="pt")
                        for j in range(4):
                            nc.tensor.transpose(
                                pt[:, j, :],
                                sTt[:, (gt * 4 + j) * 128:(gt * 4 + j + 1) * 128],
                                identb[:, :],
                            )
                        nc.vector.tensor_copy(
                            sres[:, gt * 4:(gt + 1) * 4, :], pt[:, :, :]
                        )

                    for g in range(G):
                        ps = spsum.tile([128, 512], F32, tag="ps")
                        for d in range(DF):
                            nc.tensor.matmul(
                                ps[:, :],
                                w1bf[:, d, :],
                                xst[:, g, d, :],
                                start=(d == 0), stop=(d == DF - 1),
                            )
                        nc.vector.tensor_copy(
                            sTt[:, g * 512:(g + 1) * 512], ps[:, :]
                        )
                        if g >= 1:
                            transpose_group(g - 1)
                        if g >= LAG + 1:
                            for k in range(4 * (g - LAG - 1), 4 * (g - LAG)):
                                l1_chunk(k)

                    transpose_group(G - 1)
                    for k in range(4 * (G - LAG - 1), K):
                        l1_chunk(k)

                    for mh in range(MH):
                        nc.scalar.activation(
                            hTt[:, mh * MW:(mh + 1) * MW], ph[mh][:, :],
                            mybir.ActivationFunctionType.Relu,
                            bias=b1sb[:, 0:1], scale=1.0,
                        )

            # ---- z_t = (h_t @ W2)/16 as fp8, AllGather ----
            with tc.tile_pool(name="zpsum", bufs=2, space="PSUM") as zp:
                for a in range(A):
                    pz = zp.tile([128, NCLASS], F32, tag="pz")
                    nc.tensor.matmul(
                        pz[:, :],
                        hTt[:, a * 128:(a + 1) * 128],
                        w2bf[:, :],
                        start=True, stop=True,
                    )
                    nc.scalar.activation(
                        zloc[:, a, :], pz[:, :],
                        mybir.ActivationFunctionType.Copy,
                        bias=0.0, scale=1.0 / ZSCALE,
                    )
            nc.gpsimd.dma_start(
                out=z_in.rearrange("p (a c) -> p a c", a=A), in_=zloc[:, :, :]
            )
            nc.gpsimd.collective_compute(
                "AllGather", mybir.AluOpType.bypass, replica_groups=rg,
                ins=[z_in[:, :]], outs=[z_out[:, :]],
            )

            zrf = zres.rearrange("p k i c -> p (k i) c")
            zov = z_out.rearrange("(j p) (a c) -> p j a c", p=128, a=A)
            nc.sync.dma_start(
                out=zrf[:, 0:K // 2, :].rearrange("p (j a) c -> p j a c", a=A),
                in_=zov[:, 0:NCORES // 2, :, :],
            )
            nc.scalar.dma_start(
                out=zrf[:, K // 2:, :].rearrange("p (j a) c -> p j a c", a=A),
                in_=zov[:, NCORES // 2:, :, :],
            )

            # ---- layer 2 (fp8 DoubleRow, 2 concurrent column groups) +
            # log_softmax, split per mh half so the vector/scalar softmax
            # work overlaps the other half's matmuls. ----
            with (
                tc.tile_pool(name="lpsum", bufs=1, space="PSUM") as lp,
                tc.tile_pool(name="smp", bufs=1, space="PSUM") as smp,
                tc.tile_pool(name="sms", bufs=1) as sms,
            ):
                ptrs = smp.tile([128, MC, NCLASS], F32)
                lttmp = sms.tile([NCLASS, MW], F32)
                mx = sms.tile([128, MC], F32)
                ssum = sms.tile([128, MC], F32)
                lse = sms.tile([128, MC], F32)
                bias2 = sms.tile([128, MC], F32)
                esc = sms.tile([128, MC, NCLASS], F32)
                MCH = MC // MH                 # 128-row chunks per half
                for mh in range(MH):
                    pl = lp.tile([128, MW], F32, tag="pl")
                    for k2 in range(K2):
                        # fp8 moving, two concurrent PE column groups
                        nc.tensor.matmul(
                            pl[0:NCLASS, :],
                            zres[:, k2, 0, :],
                            adjres[:, k2, 0, mh * MW:(mh + 1) * MW],
                            start=(k2 == 0), stop=(k2 == K2 - 1),
                            tile_position=(0, 0),
                            skip_group_check=True,
                        )
                        nc.tensor.matmul(
                            pl[64:64 + NCLASS, :],
                            zres[:, k2, 1, :],
                            adjres[:, k2, 1, mh * MW:(mh + 1) * MW],
                            start=(k2 == 0), stop=(k2 == K2 - 1),
                            tile_position=(0, 64),
                            skip_group_check=True,
                        )
                    nc.scalar.activation(
                        lTsb[:, mh * MW:(mh + 1) * MW], pl[64:64 + NCLASS, :],
                        mybir.ActivationFunctionType.Identity,
                        bias=b2sb[:, 0:1], scale=ZSCALE,
                    )
                    nc.scalar.activation(
                        lttmp[:, :], pl[0:NCLASS, :],
                        mybir.ActivationFunctionType.Copy,
                        bias=0.0, scale=ZSCALE,
                    )
                    nc.vector.tensor_tensor(
                        lTsb[:, mh * MW:(mh + 1) * MW],
                        lTsb[:, mh * MW:(mh + 1) * MW], lttmp[:, :],
                        op=mybir.AluOpType.add,
                    )
                    for mc in range(mh * MCH, (mh + 1) * MCH):
                        nc.tensor.transpose(
                            ptrs[:, mc, :], lTsb[:, mc * 128:(mc + 1) * 128],
                            ident[0:NCLASS, 0:NCLASS],
                        )
                        nc.vector.tensor_reduce(
                            mx[:, mc:mc + 1], ptrs[:, mc, :],
                            axis=mybir.AxisListType.X,
                            op=mybir.AluOpType.max, negate=True,
                        )
                        nc.scalar.activation(
                            esc[:, mc, :], ptrs[:, mc, :],
                            mybir.ActivationFunctionType.Exp,
                            bias=mx[:, mc:mc + 1], scale=1.0,
                            accum_out=ssum[:, mc:mc + 1],
                        )
                nc.scalar.activation(
                    lse[:, :], ssum[:, :], mybir.ActivationFunctionType.Ln,
                )
                nc.vector.tensor_sub(bias2[:, :], mx[:, :], lse[:, :])
                for mc in range(MC):
                    nc.scalar.activation(
                        osb[:, mc, :], ptrs[:, mc, :],
                        mybir.ActivationFunctionType.Identity,
                        bias=bias2[:, mc:mc + 1], scale=1.0,
                    )
            # contiguous per-partition lines; host reorders. Split in two so
            # the first half overlaps the second half's epilogue.
            oview = out_ext.ap().rearrange("p (a c) -> p a c", a=MC)
            nc.sync.dma_start(
                out=oview[:, 0:MC // 2, :], in_=osb[:, 0:MC // 2, :]
            )
            nc.sync.dma_start(
                out=oview[:, MC // 2:, :], in_=osb[:, MC // 2:, :]
            )

    nc.compile()
    return nc


_NC_CACHE = {}


def _get_nc(n_total: int):
    if n_total not in _NC_CACHE:
        _NC_CACHE[n_total] = build(n_total)
    return _NC_CACHE[n_total]


def make_in_maps(x, adj, W1, b1, W2, b2):
    n_total = x.shape[0]
    m = n_total // NCORES
    g = n_total // 512
    k2 = n_total // 256
    # xb[p, g, d, m'] = x^T[d*128+p, g*512+m']  (replicated, bf16)
    xT = np.ascontiguousarray(x.T.astype(ml_dtypes.bfloat16))
    xbp = np.ascontiguousarray(
        xT.reshape(DFG := NFEAT // 128, 128, g, 512).transpose(1, 2, 0, 3)
    ).reshape(128, g * DFG * 512)
    in_maps = []
    for t in range(NCORES):
        c0 = t * m
        # adjp[p, k2, i, m] = adj[(k2*2+i)*128+p, c0+m]  fp8
        asl = adj[:, c0:c0 + m].astype(ml_dtypes.float8_e4m3)
        ap8 = np.ascontiguousarray(
            asl.reshape(k2, 2, 128, m).transpose(2, 0, 1, 3)
        ).reshape(128, k2 * 2 * m)
        in_maps.append({
            "xb": xbp,
            "adjp": ap8,
            "w1": np.ascontiguousarray(W1),
            "b1": np.ascontiguousarray(b1.reshape(NHID, 1)),
            "w2": np.ascontiguousarray(W2),
            "b2": np.ascontiguousarray(b2.reshape(NCLASS, 1)),
        })
    return in_maps


def _assemble(res_list):
    """[128, MC*NCLASS] per core -> [N, NCLASS]."""
    outs = []
    for r in res_list:
        o = np.asarray(r["out"])
        mc = o.shape[1] // NCLASS
        outs.append(
            o.reshape(128, mc, NCLASS).transpose(1, 0, 2).reshape(-1, NCLASS)
        )
    return np.concatenate(outs, axis=0)


def kernel(x, adj, W1, b1, W2, b2):
    x = np.asarray(x, dtype=np.float32)
    adj = np.asarray(adj, dtype=np.float32)
    W1 = np.asarray(W1, dtype=np.float32)
    b1 = np.asarray(b1, dtype=np.float32)
    W2 = np.asarray(W2, dtype=np.float32)
    b2 = np.asarray(b2, dtype=np.float32)
    nc = _get_nc(x.shape[0])
    in_maps = make_in_maps(x, adj, W1, b1, W2, b2)
    res = run_bass_kernel_spmd(nc, in_maps, list(range(NCORES)))
    return _assemble([res.results[i] for i in range(NCORES)])
